# revision 1
# baseline (speedup 1.0000x reference)
"""Trainium2 Bass kernel for weighted-KDE log-density (retrieval_knn).

Math:
  out[b] = logsumexp_n( -||x_b - X_n||^2 / (2 bw^2) + log_softmax(W)_n ) + log_norm
         = logsumexp_n( 100 x_b . X_n + c_n ) + hterm_b
  with bw = 0.1,
  c_n = log_softmax(W)_n - 50 ||X_n||^2,
  hterm_b = -50 ||x_b||^2 - (d/2) log(2 pi bw^2).

Device strategy (8 cores, data-parallel over the 8192-query batch):
  * 1024 queries per core, as 8 partition-tiles of 128.
  * fp16 matmul (100 x)^T tiles against X^T chunks, fp32 PSUM accumulate;
    the per-point bias c is accumulated into the same PSUM tile by a K=2
    ones-matmul against an fp16 hi/lo split of c (keeps bias error ~1e-3).
  * Per 2048-wide n-chunk: VectorE tensor_reduce takes the chunk max
    (negated) straight from PSUM; ScalarE Exp activation with per-partition
    bias (-chunk max) and accum_out produces the chunk's sum of exps in one
    PSUM pass. No full-size intermediate is materialized in SBUF.
  * Device emits per-(query, chunk) pairs (-max, sumexp); host combines the
    8 chunks per query in float64 (exact logsumexp merge) and adds hterm.
"""

import numpy as np

B, N, D = 8192, 16384, 256
BW = 0.1
NCORES = 8
BLOC = B // NCORES            # 1024 queries per core
P = 128
NBT = BLOC // P               # 8 b-tiles per core
CHUNK = 1024
NCH = N // CHUNK              # n-chunks (16)
NF = 512                      # matmul free-dim slice

_prog_cache = {}

# ---------------------------------------------------------------------------
# Workaround: this walrus build rejects instructions carrying more than one
# sync wait ("Too many sync wait commands"). Tile attaches multi-waits to
# instructions. Split them at the BIR-JSON level: move all but the last wait
# of an instruction onto same-engine NoOps inserted just before it.
# ---------------------------------------------------------------------------
_patched = [False]


def _split_multiwaits_json(bir: bytes) -> bytes:
    import json

    d = json.loads(bir)
    uid = [0]
    for fn in d.get("functions", []):
        for blk in fn.get("blocks", []):
            insts = blk.get("instructions", [])
            out = []
            for inst in insts:
                si = inst.get("sync_info")
                waits = si.get("on_wait", []) if si else []
                if len(waits) > 1:
                    for w in waits[:-1]:
                        uid[0] += 1
                        out.append({
                            "debug": inst.get("debug", 0),
                            "engine": inst["engine"],
                            "ins": [],
                            "name": f"{inst['name']}_wsplit{uid[0]}",
                            "opcode": "NoOp",
                            "outs": [],
                            "sync_info": {"on_update": [], "on_wait": [w]},
                        })
                    si["on_wait"] = [waits[-1]]
                out.append(inst)
            blk["instructions"] = out
    return json.dumps(d).encode()


def _apply_patch():
    if _patched[0]:
        return
    from concourse import bass_utils, bass2jax

    orig = bass_utils.compile_bir_kernel

    def wrapped(bir_json, tmpdir, neff_name="file.neff"):
        return orig(_split_multiwaits_json(bir_json), tmpdir, neff_name=neff_name)

    bass_utils.compile_bir_kernel = wrapped
    if getattr(bass2jax, "compile_bir_kernel", None) is orig:
        bass2jax.compile_bir_kernel = wrapped
    _patched[0] = True


# ---------------------------------------------------------------------------


def _build_program():
    import concourse.bass as bass
    import concourse.tile as tile
    from concourse import mybir

    f16 = mybir.dt.float16
    f32 = mybir.dt.float32
    Alu = mybir.AluOpType
    Act = mybir.ActivationFunctionType

    nc = bass.Bass("TRN2", target_bir_lowering=False, debug=False)

    xT = nc.dram_tensor("xT", [2, P, BLOC], f16, kind="ExternalInput").ap()
    XT = nc.dram_tensor("XT", [2, P, N], f16, kind="ExternalInput").ap()
    c2 = nc.dram_tensor("c2", [2, N], f16, kind="ExternalInput").ap()
    on2 = nc.dram_tensor("on2", [2, P], f16, kind="ExternalInput").ap()
    res = nc.dram_tensor("res", [P, NBT * 2 * NCH], f32, kind="ExternalOutput").ap()

    with tile.TileContext(nc) as tc:
        with (
            tc.tile_pool(name="xw", bufs=1) as xw_pool,
            tc.tile_pool(name="Xc", bufs=3) as Xc_pool,
            tc.tile_pool(name="cc", bufs=3) as cc_pool,
            tc.tile_pool(name="ps", bufs=4, space="PSUM") as ps_pool,
            tc.tile_pool(name="misc", bufs=1) as misc_pool,
        ):
            xw = []
            for h in range(2):
                t_ = xw_pool.tile([P, BLOC], f16, tag=f"xw{h}")
                nc.sync.dma_start(t_[:], xT[h])
                xw.append(t_)
            ones2 = misc_pool.tile([2, P], f16, tag="ones2")
            nc.sync.dma_start(ones2[:], on2[:])

            resT = misc_pool.tile([P, NBT * 2 * NCH], f32, tag="res")
            dummy = misc_pool.tile([P, 1], f32, tag="dummy")

            for j in range(NCH):
                Xc = []
                for h in range(2):
                    t_ = Xc_pool.tile([P, CHUNK], f16, tag=f"Xc{h}")
                    nc.sync.dma_start(t_[:], XT[h][:, j * CHUNK:(j + 1) * CHUNK])
                    Xc.append(t_)
                ct = cc_pool.tile([2, CHUNK], f16, tag="ct")
                nc.sync.dma_start(ct[:], c2[:, j * CHUNK:(j + 1) * CHUNK])

                for t in range(NBT):
                    ps = ps_pool.tile([P, CHUNK], f32, tag="ps")
                    nsl = CHUNK // NF
                    for nf in range(nsl):
                        sl = slice(nf * NF, (nf + 1) * NF)
                        nc.tensor.matmul(
                            ps[:, sl], xw[0][:, t * P:(t + 1) * P], Xc[0][:, sl],
                            start=True, stop=False,
                        )
                    for nf in range(nsl):
                        sl = slice(nf * NF, (nf + 1) * NF)
                        nc.tensor.matmul(
                            ps[:, sl], xw[1][:, t * P:(t + 1) * P], Xc[1][:, sl],
                            start=False, stop=False,
                        )
                    for nf in range(nsl):
                        sl = slice(nf * NF, (nf + 1) * NF)
                        nc.tensor.matmul(
                            ps[:, sl], ones2[:], ct[:, sl],
                            start=False, stop=True,
                        )
                    stride = 2 * NCH
                    negm = resT[:, t * stride + j: t * stride + j + 1]
                    s_out = resT[:, t * stride + NCH + j: t * stride + NCH + j + 1]
                    # negm = -max over the chunk of (S + c)
                    nc.vector.tensor_reduce(
                        negm, ps[:], axis=mybir.AxisListType.X,
                        op=Alu.max, negate=True,
                    )
                    # exp(ps + negm) = exp(S + c - max); s_out = chunk sumexp
                    nc.scalar.activation(
                        dummy.broadcast_to((P, CHUNK)), ps[:], Act.Exp,
                        bias=negm, scale=1.0, accum_out=s_out,
                    )

            nc.sync.dma_start(res[:], resT[:])

    return nc


def _host_prep(x, X, W):
    x64 = np.asarray(x, dtype=np.float64)
    X64 = np.asarray(X, dtype=np.float64)
    W64 = np.asarray(W, dtype=np.float64)

    wmax = W64.max()
    logZ = np.log(np.exp(W64 - wmax).sum()) + wmax
    c = (W64 - logZ) - 50.0 * np.einsum("nd,nd->n", X64, X64)
    log_norm = -(D / 2.0) * np.log(2.0 * np.pi * BW * BW)
    hterm = -50.0 * np.einsum("bd,bd->b", x64, x64) + log_norm

    XT_f16 = np.ascontiguousarray(
        np.asarray(X, dtype=np.float32).T.astype(np.float16).reshape(2, P, N)
    )
    c_hi = c.astype(np.float16)
    c_lo = (c - c_hi.astype(np.float64)).astype(np.float16)
    c2 = np.ascontiguousarray(np.stack([c_hi, c_lo], axis=0))   # [2, N] f16
    on2 = np.ones((2, P), dtype=np.float16)
    xs = (100.0 * np.asarray(x, dtype=np.float32)).astype(np.float16)

    in_maps = []
    for k in range(NCORES):
        xk = xs[k * BLOC:(k + 1) * BLOC]          # [BLOC, D]
        xTk = np.ascontiguousarray(xk.T.reshape(2, P, BLOC))
        in_maps.append({"xT": xTk, "XT": XT_f16, "c2": c2, "on2": on2})
    return in_maps, hterm


def _host_combine(results, hterm):
    out = np.empty(B, dtype=np.float64)
    for k in range(NCORES):
        r = results[k]["res"].astype(np.float64)
        r = r.reshape(P, NBT, 2 * NCH)
        m = -r[:, :, 0:NCH]                        # [P, NBT, NCH] chunk maxes
        s = r[:, :, NCH:2 * NCH]                   # [P, NBT, NCH] chunk sumexp
        M = m.max(axis=2, keepdims=True)
        tot = np.sum(s * np.exp(m - M), axis=2)    # [P, NBT]
        lse = M[:, :, 0] + np.log(tot)             # [P, NBT]
        # query index: b = k*BLOC + t*P + p
        out[k * BLOC:(k + 1) * BLOC] = lse.T.reshape(BLOC)
    return (out + hterm).astype(np.float32)


def kernel(x, X, W, _trace=False):
    _apply_patch()
    from concourse.bass_utils import run_bass_kernel_spmd

    if "nc" not in _prog_cache:
        _prog_cache["nc"] = _build_program()
    nc = _prog_cache["nc"]

    in_maps, hterm = _host_prep(x, X, W)
    br = run_bass_kernel_spmd(
        nc, in_maps, list(range(NCORES)), trace=_trace,
    )
    kernel.last_results = br
    return _host_combine(br.results, hterm)


kernel.last_results = None



# revision 4
# speedup vs baseline: 1.4455x; 1.4455x over previous
"""Trainium2 Bass kernel for weighted-KDE log-density (retrieval_knn).

Math:
  out[b] = logsumexp_n( -50*||x_b - X_n||^2 + lgn + logsm(W)_n )
  With a = x_b - 0.5, bX = X_n - 0.5:
    out[b] = h_b + logsumexp_n( 100*a.bX_n + c_n )
    c_n = -50*||bX_n||^2 + logsm(W)_n,  h_b = -50*||a||^2 + lgn.
  The kernel bandwidth (0.1) makes the lse totally max-dominated
  (measured lse-max gap <= 0.67 vs |out| ~ 1.2e4, rel tol 2e-2), so the
  device computes PER-GRANULE MAXES only; the host merges granule maxes
  with an exact-granule-level logsumexp correction.

Device strategy (8 cores, data-parallel over the 8192-query batch):
  * u_n = 20a . bX_n(fp8) + (c_n - cbar)/5 computed in PSUM by fp8e4
    DoubleRow matmuls: one DR matmul contracts all D=256 (two K=128
    groups) at 0.5 cycles/row; the bias rides a K=1 DR matmul whose two
    rows are an fp8 hi/lo split of (c-cbar)/5.
  * Granule = 1024 points in a [128,1024] PSUM tile (2 banks), 4 tiles
    rotating. Granule pairs: ACT copies the odd granule PSUM->SBUF;
    DVE tensor_tensor_scan (op0=max, op1=max) runs over (even PSUM,
    copied SBUF) with a stride-0 broadcast output column, leaving the
    pair max in that column (2 elems/cycle on DVE, the absorb floor).
  * Host: v = 5*m + cbar + h; out = max + granule-level lse correction.
"""

import numpy as np

B, N, D = 8192, 16384, 256
BW = 0.1
NCORES = 8
BLOC = B // NCORES            # 1024 queries per core
P = 128
NBT = BLOC // P               # 8 b-tiles per core
GR = 1024                     # points per granule
NGR = N // GR                 # 16 granules per b-tile
NPAIR = NGR // 2              # 8 pair-maxes per b-tile
LAM = 5.0                     # v = LAM * u
XS = 100.0 / LAM              # x-side scale (20)

_prog_cache = {}

# ---------------------------------------------------------------------------
# Workaround: this walrus build rejects instructions carrying more than one
# sync wait ("Too many sync wait commands"). Tile attaches multi-waits to
# instructions. Split them at the BIR-JSON level: move all but the last wait
# of an instruction onto same-engine NoOps inserted just before it.
# ---------------------------------------------------------------------------
_patched = [False]


def _split_multiwaits_json(bir: bytes) -> bytes:
    import json

    d = json.loads(bir)
    uid = [0]
    for fn in d.get("functions", []):
        for blk in fn.get("blocks", []):
            insts = blk.get("instructions", [])
            out = []
            for inst in insts:
                si = inst.get("sync_info")
                waits = si.get("on_wait", []) if si else []
                if len(waits) > 1:
                    for w in waits[:-1]:
                        uid[0] += 1
                        out.append({
                            "debug": inst.get("debug", 0),
                            "engine": inst["engine"],
                            "ins": [],
                            "name": f"{inst['name']}_wsplit{uid[0]}",
                            "opcode": "NoOp",
                            "outs": [],
                            "sync_info": {"on_update": [], "on_wait": [w]},
                        })
                    si["on_wait"] = [waits[-1]]
                out.append(inst)
            blk["instructions"] = out
    return json.dumps(d).encode()


def _apply_patch():
    if _patched[0]:
        return
    from concourse import bass_utils, bass2jax

    orig = bass_utils.compile_bir_kernel

    def wrapped(bir_json, tmpdir, neff_name="file.neff"):
        return orig(_split_multiwaits_json(bir_json), tmpdir, neff_name=neff_name)

    bass_utils.compile_bir_kernel = wrapped
    if getattr(bass2jax, "compile_bir_kernel", None) is orig:
        bass2jax.compile_bir_kernel = wrapped
    _patched[0] = True


# ---------------------------------------------------------------------------


def _build_program():
    import concourse.bass as bass
    import concourse.tile as tile
    from concourse import mybir

    f8 = mybir.dt.float8e4
    f32 = mybir.dt.float32
    Alu = mybir.AluOpType
    PM = mybir.MatmulPerfMode

    nc = bass.Bass("TRN2", target_bir_lowering=False, debug=False)

    xT = nc.dram_tensor("xT", [P, 2, BLOC], f8, kind="ExternalInput").ap()
    XT = nc.dram_tensor("XT", [P, 2, N], f8, kind="ExternalInput").ap()
    c2 = nc.dram_tensor("c2", [1, 2, N], f8, kind="ExternalInput").ap()
    on2 = nc.dram_tensor("on2", [1, 2, P], f8, kind="ExternalInput").ap()
    res = nc.dram_tensor("res", [P, NBT * NPAIR], f32, kind="ExternalOutput").ap()

    with tile.TileContext(nc) as tc:
        with (
            tc.tile_pool(name="stat", bufs=1) as stat_pool,
            tc.tile_pool(name="cp", bufs=3) as cp_pool,
            tc.tile_pool(name="ps", bufs=4, space="PSUM") as ps_pool,
        ):
            xw = stat_pool.tile([P, 2, BLOC], f8, tag="xw")
            Xw = stat_pool.tile([P, 2, N], f8, tag="Xw")
            cw = stat_pool.tile([1, 2, N], f8, tag="cw")
            ow = stat_pool.tile([1, 2, P], f8, tag="ow")
            nc.sync.dma_start(xw[:], xT[:])
            nc.sync.dma_start(Xw[:], XT[:])
            nc.sync.dma_start(cw[:], c2[:])
            nc.sync.dma_start(ow[:], on2[:])

            mxt = stat_pool.tile([P, NBT * NPAIR], f32, tag="mxt")

            def emit_granule(t, g):
                """PE: u for granule g of b-tile t into a fresh psum tile."""
                ps = ps_pool.tile([P, GR], f32, tag="ps")
                lhs = xw[:, :, t * P:(t + 1) * P]
                n0 = g * GR
                for j in range(2):
                    sl = slice(j * 512, (j + 1) * 512)
                    nc.tensor.matmul(
                        ps[:, sl], lhs, Xw[:, :, n0 + j * 512:n0 + (j + 1) * 512],
                        start=True, stop=False, perf_mode=PM.DoubleRow,
                        skip_group_check=True,
                    )
                for j in range(2):
                    sl = slice(j * 512, (j + 1) * 512)
                    nc.tensor.matmul(
                        ps[:, sl], ow[:], cw[:, :, n0 + j * 512:n0 + (j + 1) * 512],
                        start=False, stop=True, perf_mode=PM.DoubleRow,
                        skip_group_check=True,
                    )
                return ps

            for t in range(NBT):
                for p in range(NPAIR):
                    psA = emit_granule(t, 2 * p)
                    psB = emit_granule(t, 2 * p + 1)
                    sbB = cp_pool.tile([P, GR], f32, tag="sbB")
                    nc.scalar.copy(sbB[:], psB[:])
                    col = t * NPAIR + p
                    nc.vector.tensor_tensor_scan(
                        mxt[:, col:col + 1].broadcast_to((P, GR)),
                        psA[:], sbB[:], -3.0e38,
                        op0=Alu.max, op1=Alu.max,
                    )

            nc.sync.dma_start(res[:], mxt[:])

    return nc


def _host_prep(x, X, W):
    import ml_dtypes

    f8 = ml_dtypes.float8_e4m3
    x64 = np.asarray(x, dtype=np.float64)
    X64 = np.asarray(X, dtype=np.float64)
    W64 = np.asarray(W, dtype=np.float64)

    a = x64 - 0.5                                   # [B, D]
    bX = X64 - 0.5                                  # [N, D]
    wmax = W64.max()
    logZ = np.log(np.exp(W64 - wmax).sum()) + wmax
    c = (W64 - logZ) - 50.0 * np.einsum("nd,nd->n", bX, bX)
    cbar = c.mean()
    cu = (c - cbar) / LAM
    c_hi = cu.astype(f8)
    c_lo = (cu - c_hi.astype(np.float64)).astype(f8)
    c2 = np.ascontiguousarray(
        np.stack([c_hi, c_lo], axis=0).reshape(1, 2, N))

    lgn = -(D / 2.0) * np.log(2.0 * np.pi * BW * BW)
    hterm = -50.0 * np.einsum("bd,bd->b", a, a) + lgn

    # XT[k, i, n] = bX[n, k + 128*i]
    XT = np.ascontiguousarray(
        bX.T.reshape(2, P, N).transpose(1, 0, 2).astype(np.float32)).astype(f8)
    on2 = np.ones((1, 2, P), dtype=np.float32).astype(f8)

    xs = (XS * a).astype(np.float32).astype(f8)     # [B, D] fp8 of 20a
    in_maps = []
    for k in range(NCORES):
        xk = xs[k * BLOC:(k + 1) * BLOC]            # [BLOC, D]
        # xT[k_, i, m] = xs[m, k_ + 128*i]
        xTk = np.ascontiguousarray(
            xk.astype(np.float32).T.reshape(2, P, BLOC).transpose(1, 0, 2)
        ).astype(f8)
        in_maps.append({"xT": xTk, "XT": XT, "c2": c2, "on2": on2})
    return in_maps, hterm, cbar


def _host_combine(results, hterm, cbar):
    out = np.empty(B, dtype=np.float64)
    for k in range(NCORES):
        m = results[k]["res"].astype(np.float64)    # [P, NBT*NPAIR]
        m = m.reshape(P, NBT, NPAIR)
        v = LAM * m + cbar                          # granule-pair max of (100 a.bX + c)
        vmax = v.max(axis=2)
        corr = np.log(np.exp(v - vmax[:, :, None]).sum(axis=2))
        lse = vmax + corr                           # [P, NBT]
        # query index: b = k*BLOC + t*P + p_row
        out[k * BLOC:(k + 1) * BLOC] = lse.T.reshape(BLOC)
    return (out + hterm).astype(np.float32)


def kernel(x, X, W, _trace=False):
    _apply_patch()
    from concourse.bass_utils import run_bass_kernel_spmd

    if "nc" not in _prog_cache:
        _prog_cache["nc"] = _build_program()
    nc = _prog_cache["nc"]

    in_maps, hterm, cbar = _host_prep(x, X, W)
    br = run_bass_kernel_spmd(
        nc, in_maps, list(range(NCORES)), trace=_trace,
    )
    kernel.last_results = br
    return _host_combine(br.results, hterm, cbar)


kernel.last_results = None


# revision 6
# speedup vs baseline: 1.5884x; 1.0989x over previous
"""Trainium2 Bass kernel for weighted-KDE log-density (retrieval_knn).

Math:
  out[b] = logsumexp_n( -50*||x_b - X_n||^2 + lgn + logsm(W)_n )
  With a = x_b - 0.5, bX = X_n - 0.5:
    out[b] = h_b + logsumexp_n( 100*a.bX_n + c_n )
    c_n = -50*||bX_n||^2 + logsm(W)_n,  h_b = -50*||a||^2 + lgn.
  The kernel bandwidth (0.1) makes the lse totally max-dominated
  (measured lse-max gap <= 0.67 vs |out| ~ 1.2e4, rel tol 2e-2), so the
  device computes PER-GRANULE MAXES only; the host merges granule maxes
  with an exact-granule-level logsumexp correction.

Device strategy (8 cores, data-parallel over the 8192-query batch):
  * u_n = 20a . bX_n(fp8) + (c_n - cbar)/5 computed in PSUM by fp8e4
    DoubleRow matmuls: one DR matmul contracts all D=256 (two K=128
    groups) at 0.5 cycles/row; the bias rides a K=1 DR matmul whose two
    rows are an fp8 hi/lo split of (c-cbar)/5.
  * Granule = 1024 points in a [128,1024] PSUM tile (2 banks), 4 tiles
    rotating. Granule pairs: ACT copies the odd granule PSUM->SBUF;
    DVE tensor_tensor_scan (op0=max, op1=max) runs over (even PSUM,
    copied SBUF) with a stride-0 broadcast output column, leaving the
    pair max in that column (2 elems/cycle on DVE, the absorb floor).
  * Host: v = 5*m + cbar + h; out = max + granule-level lse correction.
"""

import numpy as np

B, N, D = 8192, 16384, 256
BW = 0.1
NCORES = 8
BLOC = B // NCORES            # 1024 queries per core
P = 128
NBT = BLOC // P               # 8 b-tiles per core
GR = 1024                     # points per granule
NGR = N // GR                 # 16 granules per b-tile
NPAIR = NGR // 2              # 8 pair-maxes per b-tile
LAM = 5.0                     # v = LAM * u
XS = 100.0 / LAM              # x-side scale (20)

_prog_cache = {}

# ---------------------------------------------------------------------------
# Workaround: this walrus build rejects instructions carrying more than one
# sync wait ("Too many sync wait commands"). Tile attaches multi-waits to
# instructions. Split them at the BIR-JSON level: move all but the last wait
# of an instruction onto same-engine NoOps inserted just before it.
# ---------------------------------------------------------------------------
_patched = [False]


def _split_multiwaits_json(bir: bytes) -> bytes:
    import json

    d = json.loads(bir)
    uid = [0]
    for fn in d.get("functions", []):
        for blk in fn.get("blocks", []):
            insts = blk.get("instructions", [])
            out = []
            for inst in insts:
                si = inst.get("sync_info")
                waits = si.get("on_wait", []) if si else []
                if len(waits) > 1:
                    for w in waits[:-1]:
                        uid[0] += 1
                        out.append({
                            "debug": inst.get("debug", 0),
                            "engine": inst["engine"],
                            "ins": [],
                            "name": f"{inst['name']}_wsplit{uid[0]}",
                            "opcode": "NoOp",
                            "outs": [],
                            "sync_info": {"on_update": [], "on_wait": [w]},
                        })
                    si["on_wait"] = [waits[-1]]
                out.append(inst)
            blk["instructions"] = out
    return json.dumps(d).encode()


def _apply_patch():
    if _patched[0]:
        return
    from concourse import bass_utils, bass2jax

    orig = bass_utils.compile_bir_kernel

    def wrapped(bir_json, tmpdir, neff_name="file.neff"):
        return orig(_split_multiwaits_json(bir_json), tmpdir, neff_name=neff_name)

    bass_utils.compile_bir_kernel = wrapped
    if getattr(bass2jax, "compile_bir_kernel", None) is orig:
        bass2jax.compile_bir_kernel = wrapped
    _patched[0] = True


# ---------------------------------------------------------------------------


def _build_program():
    import concourse.bass as bass
    import concourse.tile as tile
    from concourse import mybir

    f8 = mybir.dt.float8e4
    f32 = mybir.dt.float32
    Alu = mybir.AluOpType
    PM = mybir.MatmulPerfMode

    nc = bass.Bass("TRN2", target_bir_lowering=False, debug=False)

    xT = nc.dram_tensor("xT", [P, 2, BLOC], f8, kind="ExternalInput").ap()
    XT = nc.dram_tensor("XT", [P, 2, N], f8, kind="ExternalInput").ap()
    c2 = nc.dram_tensor("c2", [1, 2, N], f8, kind="ExternalInput").ap()
    on2 = nc.dram_tensor("on2", [1, 2, P], f8, kind="ExternalInput").ap()
    res = nc.dram_tensor("res", [P, NBT * NPAIR], f32, kind="ExternalOutput").ap()

    with tile.TileContext(nc) as tc:
        with (
            tc.tile_pool(name="stat", bufs=1) as stat_pool,
            tc.tile_pool(name="cp", bufs=3) as cp_pool,
            tc.tile_pool(name="ps", bufs=4, space="PSUM") as ps_pool,
        ):
            xw = stat_pool.tile([P, 2, BLOC], f8, tag="xw")
            Xw = stat_pool.tile([P, 2, N], f8, tag="Xw")
            cw = stat_pool.tile([1, 2, N], f8, tag="cw")
            ow = stat_pool.tile([1, 2, P], f8, tag="ow")
            nc.sync.dma_start(ow[:], on2[:])
            nc.sync.dma_start(xw[:], xT[:])
            # chunked loads: granule-pair block p only needs chunk p, so
            # compute starts as soon as the first 2048-point chunk lands.
            for p in range(NPAIR):
                sl = slice(2 * p * GR, 2 * (p + 1) * GR)
                nc.sync.dma_start(Xw[:, :, sl], XT[:, :, sl])
                nc.sync.dma_start(cw[:, :, sl], c2[:, :, sl])

            mxt = stat_pool.tile([P, NBT * NPAIR], f32, tag="mxt")

            def emit_granule(t, g):
                """PE: u for granule g of b-tile t into a fresh psum tile."""
                ps = ps_pool.tile([P, GR], f32, tag="ps")
                lhs = xw[:, :, t * P:(t + 1) * P]
                n0 = g * GR
                for j in range(2):
                    sl = slice(j * 512, (j + 1) * 512)
                    nc.tensor.matmul(
                        ps[:, sl], lhs, Xw[:, :, n0 + j * 512:n0 + (j + 1) * 512],
                        start=True, stop=False, perf_mode=PM.DoubleRow,
                        skip_group_check=True,
                    )
                for j in range(2):
                    sl = slice(j * 512, (j + 1) * 512)
                    nc.tensor.matmul(
                        ps[:, sl], ow[:], cw[:, :, n0 + j * 512:n0 + (j + 1) * 512],
                        start=False, stop=True, perf_mode=PM.DoubleRow,
                        skip_group_check=True,
                    )
                return ps

            for p in range(NPAIR):
                for t in range(NBT):
                    psA = emit_granule(t, 2 * p)
                    psB = emit_granule(t, 2 * p + 1)
                    sbB = cp_pool.tile([P, GR], f32, tag="sbB")
                    nc.scalar.copy(sbB[:], psB[:])
                    col = t * NPAIR + p
                    nc.vector.tensor_tensor_scan(
                        mxt[:, col:col + 1].broadcast_to((P, GR)),
                        psA[:], sbB[:], -3.0e38,
                        op0=Alu.max, op1=Alu.max,
                    )

            nc.sync.dma_start(res[:], mxt[:])

    return nc


def _host_prep(x, X, W):
    import ml_dtypes

    f8 = ml_dtypes.float8_e4m3
    x64 = np.asarray(x, dtype=np.float64)
    X64 = np.asarray(X, dtype=np.float64)
    W64 = np.asarray(W, dtype=np.float64)

    a = x64 - 0.5                                   # [B, D]
    bX = X64 - 0.5                                  # [N, D]
    wmax = W64.max()
    logZ = np.log(np.exp(W64 - wmax).sum()) + wmax
    c = (W64 - logZ) - 50.0 * np.einsum("nd,nd->n", bX, bX)
    cbar = c.mean()
    cu = (c - cbar) / LAM
    c_hi = cu.astype(f8)
    c_lo = (cu - c_hi.astype(np.float64)).astype(f8)
    c2 = np.ascontiguousarray(
        np.stack([c_hi, c_lo], axis=0).reshape(1, 2, N))

    lgn = -(D / 2.0) * np.log(2.0 * np.pi * BW * BW)
    hterm = -50.0 * np.einsum("bd,bd->b", a, a) + lgn

    # XT[k, i, n] = bX[n, k + 128*i]
    XT = np.ascontiguousarray(
        bX.T.reshape(2, P, N).transpose(1, 0, 2).astype(np.float32)).astype(f8)
    on2 = np.ones((1, 2, P), dtype=np.float32).astype(f8)

    xs = (XS * a).astype(np.float32).astype(f8)     # [B, D] fp8 of 20a
    in_maps = []
    for k in range(NCORES):
        xk = xs[k * BLOC:(k + 1) * BLOC]            # [BLOC, D]
        # xT[k_, i, m] = xs[m, k_ + 128*i]
        xTk = np.ascontiguousarray(
            xk.astype(np.float32).T.reshape(2, P, BLOC).transpose(1, 0, 2)
        ).astype(f8)
        in_maps.append({"xT": xTk, "XT": XT, "c2": c2, "on2": on2})
    return in_maps, hterm, cbar


def _host_combine(results, hterm, cbar):
    out = np.empty(B, dtype=np.float64)
    for k in range(NCORES):
        m = results[k]["res"].astype(np.float64)    # [P, NBT*NPAIR]
        m = m.reshape(P, NBT, NPAIR)
        v = LAM * m + cbar                          # granule-pair max of (100 a.bX + c)
        vmax = v.max(axis=2)
        corr = np.log(np.exp(v - vmax[:, :, None]).sum(axis=2))
        lse = vmax + corr                           # [P, NBT]
        # query index: b = k*BLOC + t*P + p_row
        out[k * BLOC:(k + 1) * BLOC] = lse.T.reshape(BLOC)
    return (out + hterm).astype(np.float32)


def kernel(x, X, W, _trace=False):
    _apply_patch()
    from concourse.bass_utils import run_bass_kernel_spmd

    if "nc" not in _prog_cache:
        _prog_cache["nc"] = _build_program()
    nc = _prog_cache["nc"]

    in_maps, hterm, cbar = _host_prep(x, X, W)
    br = run_bass_kernel_spmd(
        nc, in_maps, list(range(NCORES)), trace=_trace,
    )
    kernel.last_results = br
    return _host_combine(br.results, hterm, cbar)


kernel.last_results = None


# revision 7
# speedup vs baseline: 2.0281x; 1.2768x over previous
"""Trainium2 Bass kernel for weighted-KDE log-density (retrieval_knn).

Math:
  out[b] = logsumexp_n( -50*||x_b - X_n||^2 + lgn + logsm(W)_n )
  With a = x_b - 0.5, bX = X_n - 0.5:
    out[b] = h_b + logsumexp_n( 100*a.bX_n + c_n )
    c_n = -50*||bX_n||^2 + logsm(W)_n,  h_b = -50*||a||^2 + lgn.
  The kernel bandwidth (0.1) makes the lse totally max-dominated
  (measured lse-max gap <= 0.67 vs |out| ~ 1.2e4, rel tol 2e-2), so the
  device computes PER-GRANULE MAXES only; the host merges granule maxes
  with an exact-granule-level logsumexp correction.

Device strategy (8 cores, data-parallel over the 8192-query batch):
  * u_n = 20a . bX_n(fp8) + (c_n - cbar)/5 computed in PSUM by fp8e4
    DoubleRow matmuls: one DR matmul contracts all D=256 (two K=128
    groups) at 0.5 cycles/row; the bias rides a K=1 DR matmul whose two
    rows are an fp8 hi/lo split of (c-cbar)/5.
  * Granule = 1024 points in a [128,1024] PSUM tile (2 banks), 4 tiles
    rotating. Granule pairs: ACT copies the odd granule PSUM->SBUF;
    DVE tensor_tensor_scan (op0=max, op1=max) runs over (even PSUM,
    copied SBUF) with a stride-0 broadcast output column, leaving the
    pair max in that column (2 elems/cycle on DVE, the absorb floor).
  * Host: v = 5*m + cbar + h; out = max + granule-level lse correction.
"""

import numpy as np

B, N, D = 8192, 16384, 256
BW = 0.1
NCORES = 8
BLOC = B // NCORES            # 1024 queries per core
P = 128
NBT = BLOC // P               # 8 b-tiles per core
GR = 1024                     # points per granule
NGR = N // GR                 # 16 granules per b-tile
NPAIR = NGR // 2              # 8 pair-maxes per b-tile
LAM = 5.0                     # v = LAM * u
XS = 100.0 / LAM              # x-side scale (20)

_prog_cache = {}

# ---------------------------------------------------------------------------
# Workaround: this walrus build rejects instructions carrying more than one
# sync wait ("Too many sync wait commands"). Tile attaches multi-waits to
# instructions. Split them at the BIR-JSON level: move all but the last wait
# of an instruction onto same-engine NoOps inserted just before it.
# ---------------------------------------------------------------------------
_patched = [False]


def _split_multiwaits_json(bir: bytes) -> bytes:
    import json

    d = json.loads(bir)
    uid = [0]
    for fn in d.get("functions", []):
        for blk in fn.get("blocks", []):
            insts = blk.get("instructions", [])
            out = []
            for inst in insts:
                si = inst.get("sync_info")
                waits = si.get("on_wait", []) if si else []
                if len(waits) > 1:
                    for w in waits[:-1]:
                        uid[0] += 1
                        out.append({
                            "debug": inst.get("debug", 0),
                            "engine": inst["engine"],
                            "ins": [],
                            "name": f"{inst['name']}_wsplit{uid[0]}",
                            "opcode": "NoOp",
                            "outs": [],
                            "sync_info": {"on_update": [], "on_wait": [w]},
                        })
                    si["on_wait"] = [waits[-1]]
                out.append(inst)
            blk["instructions"] = out
    return json.dumps(d).encode()


def _apply_patch():
    if _patched[0]:
        return
    from concourse import bass_utils, bass2jax

    orig = bass_utils.compile_bir_kernel

    def wrapped(bir_json, tmpdir, neff_name="file.neff"):
        return orig(_split_multiwaits_json(bir_json), tmpdir, neff_name=neff_name)

    bass_utils.compile_bir_kernel = wrapped
    if getattr(bass2jax, "compile_bir_kernel", None) is orig:
        bass2jax.compile_bir_kernel = wrapped
    _patched[0] = True


# ---------------------------------------------------------------------------


def _build_program():
    import concourse.bass as bass
    import concourse.tile as tile
    from concourse import mybir

    f8 = mybir.dt.float8e4
    f32 = mybir.dt.float32
    Alu = mybir.AluOpType
    PM = mybir.MatmulPerfMode

    nc = bass.Bass("TRN2", target_bir_lowering=False, debug=False)

    xT = nc.dram_tensor("xT", [P, 2, BLOC], f8, kind="ExternalInput").ap()
    XT = nc.dram_tensor("XT", [P, 2, N], f8, kind="ExternalInput").ap()
    c2 = nc.dram_tensor("c2", [1, 2, N], f8, kind="ExternalInput").ap()
    on2 = nc.dram_tensor("on2", [1, 2, P], f8, kind="ExternalInput").ap()
    res = nc.dram_tensor("res", [P, NBT * NPAIR], f32, kind="ExternalOutput").ap()

    with tile.TileContext(nc) as tc:
        with (
            tc.tile_pool(name="stat", bufs=1) as stat_pool,
            tc.tile_pool(name="cp", bufs=3) as cp_pool,
            tc.tile_pool(name="ps", bufs=4, space="PSUM") as ps_pool,
        ):
            xw = stat_pool.tile([P, 2, BLOC], f8, tag="xw")
            Xw = stat_pool.tile([P, 2, N], f8, tag="Xw")
            cw = stat_pool.tile([1, 2, N], f8, tag="cw")
            ow = stat_pool.tile([1, 2, P], f8, tag="ow")
            nc.sync.dma_start(ow[:], on2[:])
            nc.sync.dma_start(xw[:], xT[:])
            # chunked loads: granule-pair block p only needs chunk p, so
            # compute starts as soon as the first 2048-point chunk lands.
            for p in range(NPAIR):
                sl = slice(2 * p * GR, 2 * (p + 1) * GR)
                nc.sync.dma_start(Xw[:, :, sl], XT[:, :, sl])
                nc.sync.dma_start(cw[:, :, sl], c2[:, :, sl])

            mxt = stat_pool.tile([P, NBT * NPAIR], f32, tag="mxt")

            def emit_granule(t, g):
                """PE: u for granule g of b-tile t into a fresh psum tile."""
                ps = ps_pool.tile([P, GR], f32, tag="ps")
                lhs = xw[:, :, t * P:(t + 1) * P]
                n0 = g * GR
                for j in range(2):
                    sl = slice(j * 512, (j + 1) * 512)
                    nc.tensor.matmul(
                        ps[:, sl], lhs, Xw[:, :, n0 + j * 512:n0 + (j + 1) * 512],
                        start=True, stop=False, perf_mode=PM.DoubleRow,
                        skip_group_check=True,
                    )
                for j in range(2):
                    sl = slice(j * 512, (j + 1) * 512)
                    nc.tensor.matmul(
                        ps[:, sl], ow[:], cw[:, :, n0 + j * 512:n0 + (j + 1) * 512],
                        start=False, stop=True, perf_mode=PM.DoubleRow,
                        skip_group_check=True,
                    )
                return ps

            # Software-pipelined: the DVE scan for iteration i is emitted in
            # iteration i+1, so its ACT-copied partner is ready a full period
            # ahead and DVE runs back-to-back. B is emitted before A to
            # shorten the PE->ACT->DVE chain.
            pend = None

            def drain(pend):
                psA, sbB, col = pend
                nc.vector.tensor_tensor_scan(
                    mxt[:, col:col + 1].broadcast_to((P, GR)),
                    psA[:], sbB[:], -3.0e38,
                    op0=Alu.max, op1=Alu.max,
                )

            for p in range(NPAIR):
                for t in range(NBT):
                    psB = emit_granule(t, 2 * p + 1)
                    sbB = cp_pool.tile([P, GR], f32, tag="sbB")
                    nc.scalar.copy(sbB[:], psB[:])
                    psA = emit_granule(t, 2 * p)
                    if pend is not None:
                        drain(pend)
                    pend = (psA, sbB, t * NPAIR + p)
            drain(pend)

            nc.sync.dma_start(res[:], mxt[:])

    return nc


def _host_prep(x, X, W):
    import ml_dtypes

    f8 = ml_dtypes.float8_e4m3
    x64 = np.asarray(x, dtype=np.float64)
    X64 = np.asarray(X, dtype=np.float64)
    W64 = np.asarray(W, dtype=np.float64)

    a = x64 - 0.5                                   # [B, D]
    bX = X64 - 0.5                                  # [N, D]
    wmax = W64.max()
    logZ = np.log(np.exp(W64 - wmax).sum()) + wmax
    c = (W64 - logZ) - 50.0 * np.einsum("nd,nd->n", bX, bX)
    cbar = c.mean()
    cu = (c - cbar) / LAM
    c_hi = cu.astype(f8)
    c_lo = (cu - c_hi.astype(np.float64)).astype(f8)
    c2 = np.ascontiguousarray(
        np.stack([c_hi, c_lo], axis=0).reshape(1, 2, N))

    lgn = -(D / 2.0) * np.log(2.0 * np.pi * BW * BW)
    hterm = -50.0 * np.einsum("bd,bd->b", a, a) + lgn

    # XT[k, i, n] = bX[n, k + 128*i]
    XT = np.ascontiguousarray(
        bX.T.reshape(2, P, N).transpose(1, 0, 2).astype(np.float32)).astype(f8)
    on2 = np.ones((1, 2, P), dtype=np.float32).astype(f8)

    xs = (XS * a).astype(np.float32).astype(f8)     # [B, D] fp8 of 20a
    in_maps = []
    for k in range(NCORES):
        xk = xs[k * BLOC:(k + 1) * BLOC]            # [BLOC, D]
        # xT[k_, i, m] = xs[m, k_ + 128*i]
        xTk = np.ascontiguousarray(
            xk.astype(np.float32).T.reshape(2, P, BLOC).transpose(1, 0, 2)
        ).astype(f8)
        in_maps.append({"xT": xTk, "XT": XT, "c2": c2, "on2": on2})
    return in_maps, hterm, cbar


def _host_combine(results, hterm, cbar):
    out = np.empty(B, dtype=np.float64)
    for k in range(NCORES):
        m = results[k]["res"].astype(np.float64)    # [P, NBT*NPAIR]
        m = m.reshape(P, NBT, NPAIR)
        v = LAM * m + cbar                          # granule-pair max of (100 a.bX + c)
        vmax = v.max(axis=2)
        corr = np.log(np.exp(v - vmax[:, :, None]).sum(axis=2))
        lse = vmax + corr                           # [P, NBT]
        # query index: b = k*BLOC + t*P + p_row
        out[k * BLOC:(k + 1) * BLOC] = lse.T.reshape(BLOC)
    return (out + hterm).astype(np.float32)


def kernel(x, X, W, _trace=False):
    _apply_patch()
    from concourse.bass_utils import run_bass_kernel_spmd

    if "nc" not in _prog_cache:
        _prog_cache["nc"] = _build_program()
    nc = _prog_cache["nc"]

    in_maps, hterm, cbar = _host_prep(x, X, W)
    br = run_bass_kernel_spmd(
        nc, in_maps, list(range(NCORES)), trace=_trace,
    )
    kernel.last_results = br
    return _host_combine(br.results, hterm, cbar)


kernel.last_results = None


# revision 10
# speedup vs baseline: 2.0337x; 1.0028x over previous
"""Trainium2 Bass kernel for weighted-KDE log-density (retrieval_knn).

Math:
  out[b] = logsumexp_n( -50*||x_b - X_n||^2 + lgn + logsm(W)_n )
  With a = x_b - 0.5, bX = X_n - 0.5:
    out[b] = h_b + logsumexp_n( 100*a.bX_n + c_n )
    c_n = -50*||bX_n||^2 + logsm(W)_n,  h_b = -50*||a||^2 + lgn.
  The kernel bandwidth (0.1) makes the lse totally max-dominated
  (measured lse-max gap <= 0.67 vs |out| ~ 1.2e4, rel tol 2e-2), so the
  device computes PER-GRANULE MAXES only; the host merges granule maxes
  with an exact-granule-level logsumexp correction.

Device strategy (8 cores, data-parallel over the 8192-query batch):
  * u_n = 20a . bX_n(fp8) + (c_n - cbar)/5 computed in PSUM by fp8e4
    DoubleRow matmuls: one DR matmul contracts all D=256 (two K=128
    groups) at 0.5 cycles/row; the bias rides a K=1 DR matmul whose two
    rows are an fp8 hi/lo split of (c-cbar)/5.
  * Granule = 1024 points in a [128,1024] PSUM tile (2 banks), 4 tiles
    rotating. Granule pairs: ACT copies the odd granule PSUM->SBUF;
    DVE tensor_tensor_scan (op0=max, op1=max) runs over (even PSUM,
    copied SBUF) with a stride-0 broadcast output column, leaving the
    pair max in that column (2 elems/cycle on DVE, the absorb floor).
  * Host: v = 5*m + cbar + h; out = max + granule-level lse correction.
"""

import numpy as np

B, N, D = 8192, 16384, 256
BW = 0.1
NCORES = 8
BLOC = B // NCORES            # 1024 queries per core
P = 128
NBT = BLOC // P               # 8 b-tiles per core
GR = 1024                     # points per granule
NGR = N // GR                 # 16 granules per b-tile
NPAIR = NGR // 2              # 8 pair-maxes per b-tile
LAM = 5.0                     # v = LAM * u
XS = 100.0 / LAM              # x-side scale (20)

_prog_cache = {}

# ---------------------------------------------------------------------------
# Workaround: this walrus build rejects instructions carrying more than one
# sync wait ("Too many sync wait commands"). Tile attaches multi-waits to
# instructions. Split them at the BIR-JSON level: move all but the last wait
# of an instruction onto same-engine NoOps inserted just before it.
# ---------------------------------------------------------------------------
_patched = [False]


def _split_multiwaits_json(bir: bytes) -> bytes:
    import json

    d = json.loads(bir)
    uid = [0]
    for fn in d.get("functions", []):
        for blk in fn.get("blocks", []):
            insts = blk.get("instructions", [])
            out = []
            for inst in insts:
                si = inst.get("sync_info")
                waits = si.get("on_wait", []) if si else []
                if len(waits) > 1:
                    for w in waits[:-1]:
                        uid[0] += 1
                        out.append({
                            "debug": inst.get("debug", 0),
                            "engine": inst["engine"],
                            "ins": [],
                            "name": f"{inst['name']}_wsplit{uid[0]}",
                            "opcode": "NoOp",
                            "outs": [],
                            "sync_info": {"on_update": [], "on_wait": [w]},
                        })
                    si["on_wait"] = [waits[-1]]
                out.append(inst)
            blk["instructions"] = out
    return json.dumps(d).encode()


def _apply_patch():
    if _patched[0]:
        return
    from concourse import bass_utils, bass2jax

    orig = bass_utils.compile_bir_kernel

    def wrapped(bir_json, tmpdir, neff_name="file.neff"):
        return orig(_split_multiwaits_json(bir_json), tmpdir, neff_name=neff_name)

    bass_utils.compile_bir_kernel = wrapped
    if getattr(bass2jax, "compile_bir_kernel", None) is orig:
        bass2jax.compile_bir_kernel = wrapped
    _patched[0] = True


# ---------------------------------------------------------------------------


def _build_program():
    import concourse.bass as bass
    import concourse.tile as tile
    from concourse import mybir

    f8 = mybir.dt.float8e4
    f32 = mybir.dt.float32
    Alu = mybir.AluOpType
    PM = mybir.MatmulPerfMode

    nc = bass.Bass("TRN2", target_bir_lowering=False, debug=False)

    xT = nc.dram_tensor("xT", [P, 2, BLOC], f8, kind="ExternalInput").ap()
    XT = nc.dram_tensor("XT", [P, 2, N], f8, kind="ExternalInput").ap()
    c2 = nc.dram_tensor("c2", [1, 2, N], f8, kind="ExternalInput").ap()
    on2 = nc.dram_tensor("on2", [1, 2, P], f8, kind="ExternalInput").ap()
    res = nc.dram_tensor("res", [P, NBT * NPAIR], f32, kind="ExternalOutput").ap()

    with tile.TileContext(nc) as tc:
        with (
            tc.tile_pool(name="stat", bufs=1) as stat_pool,
            tc.tile_pool(name="cp", bufs=3) as cp_pool,
            tc.tile_pool(name="ps", bufs=4, space="PSUM") as ps_pool,
        ):
            xw = stat_pool.tile([P, 2, BLOC], f8, tag="xw")
            Xw = stat_pool.tile([P, 2, N], f8, tag="Xw")
            cw = stat_pool.tile([1, 2, N], f8, tag="cw")
            ow = stat_pool.tile([1, 2, P], f8, tag="ow")
            # First wave on three different queues: the B-granule of pair 0
            # (cols GR:2GR) gates the whole PE->ACT->DVE pipeline, so its X
            # and c slices go out first from otherwise-idle engines.
            slB0 = slice(GR, 2 * GR)
            slA0 = slice(0, GR)
            nc.gpsimd.dma_start(Xw[:, :, slB0], XT[:, :, slB0])
            nc.scalar.dma_start(cw[:, :, slB0], c2[:, :, slB0])
            nc.sync.dma_start(ow[:], on2[:])
            nc.sync.dma_start(xw[:], xT[:])
            nc.sync.dma_start(Xw[:, :, slA0], XT[:, :, slA0])
            nc.sync.dma_start(cw[:, :, slA0], c2[:, :, slA0])
            # chunked loads: granule-pair block p only needs chunk p, so
            # compute starts as soon as the first 2048-point chunk lands.
            for p in range(1, NPAIR):
                sl = slice(2 * p * GR, 2 * (p + 1) * GR)
                nc.sync.dma_start(Xw[:, :, sl], XT[:, :, sl])
                nc.sync.dma_start(cw[:, :, sl], c2[:, :, sl])

            mxt = stat_pool.tile([P, NBT * NPAIR], f32, tag="mxt")

            def emit_granule(t, g):
                """PE: u for granule g of b-tile t into a fresh psum tile."""
                ps = ps_pool.tile([P, GR], f32, tag="ps")
                lhs = xw[:, :, t * P:(t + 1) * P]
                n0 = g * GR
                for j in range(2):
                    sl = slice(j * 512, (j + 1) * 512)
                    nc.tensor.matmul(
                        ps[:, sl], lhs, Xw[:, :, n0 + j * 512:n0 + (j + 1) * 512],
                        start=True, stop=False, perf_mode=PM.DoubleRow,
                        skip_group_check=True,
                    )
                for j in range(2):
                    sl = slice(j * 512, (j + 1) * 512)
                    nc.tensor.matmul(
                        ps[:, sl], ow[:], cw[:, :, n0 + j * 512:n0 + (j + 1) * 512],
                        start=False, stop=True, perf_mode=PM.DoubleRow,
                        skip_group_check=True,
                    )
                return ps

            # Software-pipelined: the DVE scan for iteration i is emitted in
            # iteration i+1, so its ACT-copied partner is ready a full period
            # ahead and DVE runs back-to-back. B is emitted before A to
            # shorten the PE->ACT->DVE chain.
            pend = None

            def drain(pend):
                psA, sbB, col = pend
                nc.vector.tensor_tensor_scan(
                    mxt[:, col:col + 1].broadcast_to((P, GR)),
                    psA[:], sbB[:], -3.0e38,
                    op0=Alu.max, op1=Alu.max,
                )

            for p in range(NPAIR):
                for t in range(NBT):
                    psB = emit_granule(t, 2 * p + 1)
                    sbB = cp_pool.tile([P, GR], f32, tag="sbB")
                    nc.scalar.copy(sbB[:], psB[:])
                    psA = emit_granule(t, 2 * p)
                    if pend is not None:
                        drain(pend)
                    pend = (psA, sbB, t * NPAIR + p)
            # all columns except the final iteration's can ship while the
            # last scan runs
            last_col = pend[2]
            nc.sync.dma_start(res[:, 0:last_col], mxt[:, 0:last_col])
            drain(pend)
            nc.sync.dma_start(res[:, last_col:last_col + 1],
                              mxt[:, last_col:last_col + 1])

    return nc


def _host_prep(x, X, W):
    import ml_dtypes

    f8 = ml_dtypes.float8_e4m3
    x64 = np.asarray(x, dtype=np.float64)
    X64 = np.asarray(X, dtype=np.float64)
    W64 = np.asarray(W, dtype=np.float64)

    a = x64 - 0.5                                   # [B, D]
    bX = X64 - 0.5                                  # [N, D]
    wmax = W64.max()
    logZ = np.log(np.exp(W64 - wmax).sum()) + wmax
    c = (W64 - logZ) - 50.0 * np.einsum("nd,nd->n", bX, bX)
    cbar = c.mean()
    cu = (c - cbar) / LAM
    c_hi = cu.astype(f8)
    c_lo = (cu - c_hi.astype(np.float64)).astype(f8)
    c2 = np.ascontiguousarray(
        np.stack([c_hi, c_lo], axis=0).reshape(1, 2, N))

    lgn = -(D / 2.0) * np.log(2.0 * np.pi * BW * BW)
    hterm = -50.0 * np.einsum("bd,bd->b", a, a) + lgn

    # XT[k, i, n] = bX[n, k + 128*i]
    XT = np.ascontiguousarray(
        bX.T.reshape(2, P, N).transpose(1, 0, 2).astype(np.float32)).astype(f8)
    on2 = np.ones((1, 2, P), dtype=np.float32).astype(f8)

    xs = (XS * a).astype(np.float32).astype(f8)     # [B, D] fp8 of 20a
    in_maps = []
    for k in range(NCORES):
        xk = xs[k * BLOC:(k + 1) * BLOC]            # [BLOC, D]
        # xT[k_, i, m] = xs[m, k_ + 128*i]
        xTk = np.ascontiguousarray(
            xk.astype(np.float32).T.reshape(2, P, BLOC).transpose(1, 0, 2)
        ).astype(f8)
        in_maps.append({"xT": xTk, "XT": XT, "c2": c2, "on2": on2})
    return in_maps, hterm, cbar


def _host_combine(results, hterm, cbar):
    out = np.empty(B, dtype=np.float64)
    for k in range(NCORES):
        m = results[k]["res"].astype(np.float64)    # [P, NBT*NPAIR]
        m = m.reshape(P, NBT, NPAIR)
        v = LAM * m + cbar                          # granule-pair max of (100 a.bX + c)
        vmax = v.max(axis=2)
        corr = np.log(np.exp(v - vmax[:, :, None]).sum(axis=2))
        lse = vmax + corr                           # [P, NBT]
        # query index: b = k*BLOC + t*P + p_row
        out[k * BLOC:(k + 1) * BLOC] = lse.T.reshape(BLOC)
    return (out + hterm).astype(np.float32)


def kernel(x, X, W, _trace=False):
    _apply_patch()
    from concourse.bass_utils import run_bass_kernel_spmd

    if "nc" not in _prog_cache:
        _prog_cache["nc"] = _build_program()
    nc = _prog_cache["nc"]

    in_maps, hterm, cbar = _host_prep(x, X, W)
    br = run_bass_kernel_spmd(
        nc, in_maps, list(range(NCORES)), trace=_trace,
    )
    kernel.last_results = br
    return _host_combine(br.results, hterm, cbar)


kernel.last_results = None


# revision 16
# speedup vs baseline: 2.0572x; 1.0116x over previous
"""Trainium2 Bass kernel for weighted-KDE log-density (retrieval_knn).

Math:
  out[b] = logsumexp_n( -50*||x_b - X_n||^2 + lgn + logsm(W)_n )
  With a = x_b - 0.5, bX = X_n - 0.5:
    out[b] = h_b + logsumexp_n( 100*a.bX_n + c_n )
    c_n = -50*||bX_n||^2 + logsm(W)_n,  h_b = -50*||a||^2 + lgn.
  The kernel bandwidth (0.1) makes the lse totally max-dominated
  (measured lse-max gap <= 0.67 vs |out| ~ 1.2e4, rel tol 2e-2), so the
  device computes PER-GRANULE MAXES only; the host merges granule maxes
  with an exact-granule-level logsumexp correction.

Device strategy (8 cores, data-parallel over the 8192-query batch):
  * u_n = 20a . bX_n(fp8) + (c_n - cbar)/5 computed in PSUM by fp8e4
    DoubleRow matmuls: one DR matmul contracts all D=256 (two K=128
    groups) at 0.5 cycles/row; the bias rides a K=1 DR matmul whose two
    rows are an fp8 hi/lo split of (c-cbar)/5.
  * Granule = 1024 points in a [128,1024] PSUM tile (2 banks), 4 tiles
    rotating. Granule pairs: ACT copies the odd granule PSUM->SBUF;
    DVE tensor_tensor_scan (op0=max, op1=max) runs over (even PSUM,
    copied SBUF) with a stride-0 broadcast output column, leaving the
    pair max in that column (2 elems/cycle on DVE, the absorb floor).
  * Host: v = 5*m + cbar + h; out = max + granule-level lse correction.
"""

import numpy as np

B, N, D = 8192, 16384, 256
BW = 0.1
NCORES = 8
BLOC = B // NCORES            # 1024 queries per core
P = 128
NBT = BLOC // P               # 8 b-tiles per core
GR = 1024                     # points per granule
NGR = N // GR                 # 16 granules per b-tile
NPAIR = NGR // 2              # 8 pair-maxes per b-tile
LAM = 5.0                     # v = LAM * u
XS = 100.0 / LAM              # x-side scale (20)

_prog_cache = {}

# ---------------------------------------------------------------------------
# Workaround: this walrus build rejects instructions carrying more than one
# sync wait ("Too many sync wait commands"). Tile attaches multi-waits to
# instructions. Split them at the BIR-JSON level: move all but the last wait
# of an instruction onto same-engine NoOps inserted just before it.
# ---------------------------------------------------------------------------
_patched = [False]


def _split_multiwaits_json(bir: bytes) -> bytes:
    import json

    d = json.loads(bir)
    uid = [0]
    for fn in d.get("functions", []):
        for blk in fn.get("blocks", []):
            insts = blk.get("instructions", [])
            out = []
            for inst in insts:
                si = inst.get("sync_info")
                waits = si.get("on_wait", []) if si else []
                if len(waits) > 1:
                    for w in waits[:-1]:
                        uid[0] += 1
                        out.append({
                            "debug": inst.get("debug", 0),
                            "engine": inst["engine"],
                            "ins": [],
                            "name": f"{inst['name']}_wsplit{uid[0]}",
                            "opcode": "NoOp",
                            "outs": [],
                            "sync_info": {"on_update": [], "on_wait": [w]},
                        })
                    si["on_wait"] = [waits[-1]]
                out.append(inst)
            blk["instructions"] = out
    return json.dumps(d).encode()


def _apply_patch():
    if _patched[0]:
        return
    from concourse import bass_utils, bass2jax

    orig = bass_utils.compile_bir_kernel

    def wrapped(bir_json, tmpdir, neff_name="file.neff"):
        return orig(_split_multiwaits_json(bir_json), tmpdir, neff_name=neff_name)

    bass_utils.compile_bir_kernel = wrapped
    if getattr(bass2jax, "compile_bir_kernel", None) is orig:
        bass2jax.compile_bir_kernel = wrapped
    _patched[0] = True


# ---------------------------------------------------------------------------


def _build_program():
    import concourse.bass as bass
    import concourse.tile as tile
    from concourse import mybir

    f8 = mybir.dt.float8e4
    f32 = mybir.dt.float32
    Alu = mybir.AluOpType
    PM = mybir.MatmulPerfMode

    nc = bass.Bass("TRN2", target_bir_lowering=False, debug=False)

    xT = nc.dram_tensor("xT", [P, 2, BLOC], f8, kind="ExternalInput").ap()
    XT = nc.dram_tensor("XT", [P, 2, N], f8, kind="ExternalInput").ap()
    c2 = nc.dram_tensor("c2", [1, 2, N], f8, kind="ExternalInput").ap()
    on2 = nc.dram_tensor("on2", [1, 2, P], f8, kind="ExternalInput").ap()
    res = nc.dram_tensor("res", [P, NBT * NPAIR + 1], f32,
                         kind="ExternalOutput").ap()

    with tile.TileContext(nc) as tc:
        with (
            tc.tile_pool(name="stat", bufs=1) as stat_pool,
            tc.tile_pool(name="cp", bufs=3) as cp_pool,
            tc.tile_pool(name="ps", bufs=4, space="PSUM") as ps_pool,
        ):
            xw = stat_pool.tile([P, 2, BLOC], f8, tag="xw")
            Xw = stat_pool.tile([P, 2, N], f8, tag="Xw")
            cw = stat_pool.tile([1, 2, N], f8, tag="cw")
            ow = stat_pool.tile([1, 2, P], f8, tag="ow")
            # First wave on three different queues: the B-granule of pair 0
            # (cols GR:2GR) gates the whole PE->ACT->DVE pipeline, so its X
            # and c slices go out first from otherwise-idle engines.
            slB0 = slice(GR, 2 * GR)
            slA0 = slice(0, GR)
            nc.gpsimd.dma_start(Xw[:, :, slB0], XT[:, :, slB0])
            nc.scalar.dma_start(cw[:, :, slB0], c2[:, :, slB0])
            nc.sync.dma_start(xw[:], xT[:])
            nc.sync.dma_start(ow[:], on2[:])
            nc.sync.dma_start(Xw[:, :, slA0], XT[:, :, slA0])
            nc.sync.dma_start(cw[:, :, slA0], c2[:, :, slA0])
            # chunked loads: granule-pair block p only needs chunk p, so
            # compute starts as soon as the first 2048-point chunk lands.
            for p in range(1, NPAIR):
                sl = slice(2 * p * GR, 2 * (p + 1) * GR)
                nc.sync.dma_start(Xw[:, :, sl], XT[:, :, sl])
                nc.sync.dma_start(cw[:, :, sl], c2[:, :, sl])

            mxt = stat_pool.tile([P, NBT * NPAIR + 1], f32, tag="mxt")

            def emit_granule(t, n0, sz):
                """PE: u for points [n0, n0+sz) of b-tile t into a psum tile."""
                ps = ps_pool.tile([P, GR], f32, tag="ps")
                lhs = xw[:, :, t * P:(t + 1) * P]
                for j in range(sz // 512):
                    sl = slice(j * 512, (j + 1) * 512)
                    nc.tensor.matmul(
                        ps[:, sl], lhs, Xw[:, :, n0 + j * 512:n0 + (j + 1) * 512],
                        start=True, stop=False, perf_mode=PM.DoubleRow,
                        skip_group_check=True,
                    )
                for j in range(sz // 512):
                    sl = slice(j * 512, (j + 1) * 512)
                    nc.tensor.matmul(
                        ps[:, sl], ow[:], cw[:, :, n0 + j * 512:n0 + (j + 1) * 512],
                        start=False, stop=True, perf_mode=PM.DoubleRow,
                        skip_group_check=True,
                    )
                return ps

            # Software-pipelined: the DVE scan for iteration i is emitted in
            # iteration i+1, so its ACT-copied partner is ready a full period
            # ahead and DVE runs back-to-back. B is emitted before A to
            # shorten the PE->ACT->DVE chain.
            pend = None

            def drain(pend):
                psA, sbB, col, sz = pend
                nc.vector.tensor_tensor_scan(
                    mxt[:, col:col + 1].broadcast_to((P, sz)),
                    psA[:, 0:sz], sbB[:, 0:sz], -3.0e38,
                    op0=Alu.max, op1=Alu.max,
                )

            # iteration list: (t, A start, B start, granule size, out col).
            # The first standard pair (p=0, t=0) is split into two 512-point
            # halves so the PE->ACT->DVE pipeline fills ~3us sooner.
            iters = [
                (0, 0, GR, 512, NBT * NPAIR),
                (0, 512, GR + 512, 512, 0),
            ]
            for p in range(NPAIR):
                for t in range(NBT):
                    if p == 0 and t == 0:
                        continue
                    iters.append((t, 2 * p * GR, (2 * p + 1) * GR, GR,
                                  t * NPAIR + p))

            for (t, nA, nB, sz, col) in iters:
                psB = emit_granule(t, nB, sz)
                sbB = cp_pool.tile([P, GR], f32, tag="sbB")
                nc.scalar.copy(sbB[:, 0:sz], psB[:, 0:sz])
                psA = emit_granule(t, nA, sz)
                if pend is not None:
                    drain(pend)
                pend = (psA, sbB, col, sz)
            # all columns except the final iteration's can ship while the
            # last scan runs
            last_col = pend[2]
            assert last_col == NBT * NPAIR - 1
            nc.sync.dma_start(res[:, 0:last_col], mxt[:, 0:last_col])
            nc.sync.dma_start(res[:, last_col + 1:], mxt[:, last_col + 1:])
            drain(pend)
            nc.sync.dma_start(res[:, last_col:last_col + 1],
                              mxt[:, last_col:last_col + 1])

    return nc


def _host_prep(x, X, W):
    import ml_dtypes

    f8 = ml_dtypes.float8_e4m3
    x64 = np.asarray(x, dtype=np.float64)
    X64 = np.asarray(X, dtype=np.float64)
    W64 = np.asarray(W, dtype=np.float64)

    a = x64 - 0.5                                   # [B, D]
    bX = X64 - 0.5                                  # [N, D]
    wmax = W64.max()
    logZ = np.log(np.exp(W64 - wmax).sum()) + wmax
    c = (W64 - logZ) - 50.0 * np.einsum("nd,nd->n", bX, bX)
    cbar = c.mean()
    cu = (c - cbar) / LAM
    c_hi = cu.astype(f8)
    c_lo = (cu - c_hi.astype(np.float64)).astype(f8)
    c2 = np.ascontiguousarray(
        np.stack([c_hi, c_lo], axis=0).reshape(1, 2, N))

    lgn = -(D / 2.0) * np.log(2.0 * np.pi * BW * BW)
    hterm = -50.0 * np.einsum("bd,bd->b", a, a) + lgn

    # XT[k, i, n] = bX[n, k + 128*i]
    XT = np.ascontiguousarray(
        bX.T.reshape(2, P, N).transpose(1, 0, 2).astype(np.float32)).astype(f8)
    on2 = np.ones((1, 2, P), dtype=np.float32).astype(f8)

    xs = (XS * a).astype(np.float32).astype(f8)     # [B, D] fp8 of 20a
    in_maps = []
    for k in range(NCORES):
        xk = xs[k * BLOC:(k + 1) * BLOC]            # [BLOC, D]
        # xT[k_, i, m] = xs[m, k_ + 128*i]
        xTk = np.ascontiguousarray(
            xk.astype(np.float32).T.reshape(2, P, BLOC).transpose(1, 0, 2)
        ).astype(f8)
        in_maps.append({"xT": xTk, "XT": XT, "c2": c2, "on2": on2})
    return in_maps, hterm, cbar


def _host_combine(results, hterm, cbar):
    out = np.empty(B, dtype=np.float64)
    for k in range(NCORES):
        m = results[k]["res"].astype(np.float64)    # [P, NBT*NPAIR + 1]
        extra = m[:, NBT * NPAIR]                   # split first-iteration col (t=0)
        m = m[:, :NBT * NPAIR].reshape(P, NBT, NPAIR)
        v = LAM * m + cbar                          # granule-pair max of (100 a.bX + c)
        ve = LAM * extra + cbar
        vmax = v.max(axis=2)
        vmax[:, 0] = np.maximum(vmax[:, 0], ve)
        corr = np.exp(v - vmax[:, :, None]).sum(axis=2)
        corr[:, 0] += np.exp(ve - vmax[:, 0])
        lse = vmax + np.log(corr)                   # [P, NBT]
        # query index: b = k*BLOC + t*P + p_row
        out[k * BLOC:(k + 1) * BLOC] = lse.T.reshape(BLOC)
    return (out + hterm).astype(np.float32)


def kernel(x, X, W, _trace=False):
    _apply_patch()
    from concourse.bass_utils import run_bass_kernel_spmd

    if "nc" not in _prog_cache:
        _prog_cache["nc"] = _build_program()
    nc = _prog_cache["nc"]

    in_maps, hterm, cbar = _host_prep(x, X, W)
    br = run_bass_kernel_spmd(
        nc, in_maps, list(range(NCORES)), trace=_trace,
    )
    kernel.last_results = br
    return _host_combine(br.results, hterm, cbar)


kernel.last_results = None
